# revision 1
# baseline (speedup 1.0000x reference)
"""Single-head attention (B=8, S=2048, D=1024, H=64) on 8 TRN2 NeuronCores.

Sharding: data-parallel over batch — one batch element per core, Q/K/V
weights replicated. No collectives; host gathers the 8 per-core outputs.

Per-core pipeline (all shapes per core):
  phase 1: x [S, D] f32 -> bf16 (GpSimd) -> PE-transpose 128x128 bf16 tiles
           -> xT; fused QKV matmul (xT stationary, W^T moving, N=192);
           bias add (DVE); q/k PE-transposed into qT/kT [H, S] bf16;
           v kept natural in v_aug [S, H+1] with a ones column.
  phase 2: scores = qT.T @ kT per 128-row q-tile (PSUM f32)
           masked bf16 tile pre-filled -300 (GpSimd memset), then
           copy_predicated(masked, mask, scores) — one DVE pass
           PE-transpose masked (bf16) -> [k, q] layout in PSUM
           probsT = exp(0.125*x) (ACT, psum->sbuf, bf16; e^-37.5 for
           masked slots ~ 5e-17)
           outT[65, q] += v_aug[k-tile].T @ probsT (PSUM accum over k;
           ones column gives the softmax denominators for free)
           PE-transpose back, multiply by reciprocal (DVE), DMA out.
"""

import sys
import types

import numpy as np

import concourse.bass as bass
import concourse.mybir as mybir
import concourse.tile as tile
from concourse import bacc
from concourse.bass_utils import run_bass_kernel_spmd
from concourse.masks import make_identity

B, S, D, H = 8, 2048, 1024, 64
NT = S // 128          # 16 seq tiles of 128
NCH = D // 128         # 8 contraction chunks
NG = 4                 # q-tile groups of 4 (512 q columns per group)
MASK_FILL = -300.0     # exp(0.125 * -300) = e^-37.5 ~ 5.2e-17

f32 = mybir.dt.float32
bf16 = mybir.dt.bfloat16
i32 = mybir.dt.int32
ACT_EXP = mybir.ActivationFunctionType.Exp


def install_ntff_hook():
    """RL-container antenv stub lacks axon_hooks; inject it so trace=True
    under axon can capture NTFF profiles. Harmless if already present."""
    if "antenv.axon_hooks" in sys.modules:
        return
    try:
        mod = types.ModuleType("antenv.axon_hooks")
        state = {"hook": None}
        mod.set_axon_ntff_profile_hook = lambda h: state.__setitem__("hook", h)
        mod.get_axon_ntff_profile_hook = lambda: state["hook"]
        sys.modules["antenv.axon_hooks"] = mod
        import antenv

        antenv.axon_hooks = mod
        from trn_agent_boot.trn_boot import _ntff_profile_via_ctypes

        mod.set_axon_ntff_profile_hook(
            _ntff_profile_via_ctypes("/opt/axon/libaxon_pjrt.so")
        )
    except Exception:
        pass


def build():
    nc = bacc.Bacc("TRN2", target_bir_lowering=False, debug=False, num_devices=8)

    x_d = nc.dram_tensor("input", [S, D], f32, kind="ExternalInput")
    m_d = nc.dram_tensor("mask", [S, S], i32, kind="ExternalInput")
    w_d = {
        n: nc.dram_tensor(n, [H, D], f32, kind="ExternalInput")
        for n in ("W_q", "W_k", "W_v")
    }
    b_d = {
        n: nc.dram_tensor(n, [H], f32, kind="ExternalInput")
        for n in ("b_q", "b_k", "b_v")
    }
    out_d = nc.dram_tensor("out", [S, H], f32, kind="ExternalOutput")

    with tile.TileContext(nc) as tc:
        with (
            tc.tile_pool(name="singles", bufs=1) as singles,
            tc.tile_pool(name="sb", bufs=2) as sb,
            tc.tile_pool(name="msk", bufs=6) as mskp,
            tc.tile_pool(name="mkin", bufs=4) as mkin,
            tc.tile_pool(name="pA", bufs=2, space="PSUM") as pA,
            tc.tile_pool(name="pB", bufs=2, space="PSUM") as pB,
            tc.tile_pool(name="pPV", bufs=2, space="PSUM") as pPV,
        ):
            # ---- constants -------------------------------------------------
            ident = singles.tile([128, 128], f32)
            make_identity(nc, ident[:])
            id_b = singles.tile([128, 128], bf16)
            make_identity(nc, id_b[:])

            bias_bc = singles.tile([128, 192], f32)
            for wi, n in enumerate(("b_q", "b_k", "b_v")):
                src = bass.AP(tensor=b_d[n], offset=0, ap=[[0, 128], [1, H]])
                nc.gpsimd.dma_start(bias_bc[:, wi * H:(wi + 1) * H], src)

            # ---- weights: W^T in bf16, laid out [128, chunk, q|k|v] -------
            wT = singles.tile([128, NCH, 192], bf16)
            for wi, n in enumerate(("W_q", "W_k", "W_v")):
                w_nat = sb.tile([H, D], f32, tag="wnat")
                nc.sync.dma_start(w_nat[:], w_d[n].ap())
                for c in range(NCH):
                    wt_ps = pA.tile([128, H], f32, tag="A")
                    nc.tensor.transpose(
                        wt_ps[:],
                        w_nat[:, c * 128:(c + 1) * 128],
                        ident[:H, :H],
                    )
                    nc.scalar.copy(wT[:, c, wi * H:(wi + 1) * H], wt_ps[:])

            # persistent activations
            qT = singles.tile([H, S], bf16)
            kT = singles.tile([H, S], bf16)
            v_aug = singles.tile([128, NT, H + 1], bf16)
            nc.gpsimd.memset(v_aug[:, :, H:H + 1], 1.0)

            # ---- phase 1: project ------------------------------------------
            for t in range(NT):
                x_t = sb.tile([128, D], f32, tag="x")
                nc.sync.dma_start(x_t[:], x_d.ap()[t * 128:(t + 1) * 128, :])
                x_bf = sb.tile([128, D], bf16, tag="xbf")
                nc.vector.tensor_copy(x_bf[:], x_t[:])

                xt_ps = pB.tile([128, D], bf16, tag="B")
                for c in range(NCH):
                    nc.tensor.transpose(
                        xt_ps[:, c * 128:(c + 1) * 128],
                        x_bf[:, c * 128:(c + 1) * 128],
                        id_b[:],
                    )
                xT_sb = sb.tile([128, NCH, 128], bf16, tag="xT")
                nc.scalar.copy(
                    xT_sb[:].rearrange("p c f -> p (c f)"), xt_ps[:]
                )

                pj_ps = pA.tile([128, 192], f32, tag="A")
                for c in range(NCH):
                    nc.tensor.matmul(
                        pj_ps[:],
                        xT_sb[:, c, :],
                        wT[:, c, :],
                        start=(c == 0),
                        stop=(c == NCH - 1),
                    )
                qkv_sb = sb.tile([128, 192], bf16, tag="qkv")
                nc.vector.tensor_add(qkv_sb[:], pj_ps[:], bias_bc[:])

                nc.scalar.copy(v_aug[:, t, 0:H], qkv_sb[:, 128:192])

                for which, dst in ((0, qT), (1, kT)):
                    tp = pA.tile([H, 128], bf16, tag="A")
                    nc.tensor.transpose(
                        tp[:], qkv_sb[:, which * H:(which + 1) * H], id_b[:]
                    )
                    nc.vector.tensor_copy(dst[:, t * 128:(t + 1) * 128], tp[:])

            # ---- phase 2: attention ----------------------------------------
            for g in range(NG):
                masked_g = []
                for qq in range(4):
                    qt = g * 4 + qq
                    mask_t = mkin.tile([128, S], i32, tag="mk")
                    nc.sync.dma_start(
                        mask_t[:], m_d.ap()[qt * 128:(qt + 1) * 128, :]
                    )
                    masked_t = mskp.tile([128, S], bf16, tag="msk")
                    nc.gpsimd.memset(masked_t[:], MASK_FILL)
                    for hf in range(2):
                        sl = slice(hf * 1024, (hf + 1) * 1024)
                        sc_ps = pA.tile([128, 1024], f32, tag="A")
                        for ch in range(2):
                            csl = slice(ch * 512, (ch + 1) * 512)
                            nc.tensor.matmul(
                                sc_ps[:, csl],
                                qT[:, qt * 128:(qt + 1) * 128],
                                kT[:, hf * 1024 + ch * 512:hf * 1024 + (ch + 1) * 512],
                                start=True,
                                stop=True,
                            )
                        nc.vector.copy_predicated(
                            masked_t[:, sl], mask_t[:, sl], sc_ps[:]
                        )
                    masked_g.append(masked_t)

                probsT = sb.tile([128, NT, 512], bf16, tag="pT")
                for kd in range(NT // 2):  # k-tile duos
                    tr_ps = pB.tile([128, 1024], bf16, tag="B")
                    for j in range(2):
                        kt = kd * 2 + j
                        for qq in range(4):
                            nc.tensor.transpose(
                                tr_ps[:, j * 512 + qq * 128:j * 512 + (qq + 1) * 128],
                                masked_g[qq][:, kt * 128:(kt + 1) * 128],
                                id_b[:],
                            )
                    nc.scalar.activation(
                        probsT[:, kd * 2:kd * 2 + 2, :].rearrange("p a b -> p (a b)"),
                        tr_ps[:],
                        ACT_EXP,
                        bias=0.0,
                        scale=0.125,
                    )

                pv_ps = pPV.tile([H + 1, 512], f32, tag="pv")
                for kt in range(NT):
                    nc.tensor.matmul(
                        pv_ps[:],
                        v_aug[:, kt, :],
                        probsT[:, kt, :],
                        start=(kt == 0),
                        stop=(kt == NT - 1),
                    )
                oT_sb = sb.tile([H + 1, 512], f32, tag="oT")
                nc.scalar.copy(oT_sb[:], pv_ps[:])

                for qq in range(4):
                    qt = g * 4 + qq
                    o2_ps = pA.tile([128, H + 1], f32, tag="A")
                    nc.tensor.transpose(
                        o2_ps[:],
                        oT_sb[:, qq * 128:(qq + 1) * 128],
                        ident[:H + 1, :H + 1],
                    )
                    rcp = sb.tile([128, 1], f32, tag="rcp")
                    nc.vector.reciprocal(rcp[:], o2_ps[:, H:H + 1])
                    out_sb = sb.tile([128, H], f32, tag="osb")
                    nc.vector.tensor_scalar_mul(
                        out_sb[:], o2_ps[:, 0:H], rcp[:]
                    )
                    nc.sync.dma_start(
                        out_d.ap()[qt * 128:(qt + 1) * 128, :], out_sb[:]
                    )

    nc.compile()
    return nc


_NC_CACHE = None


def _get_nc():
    global _NC_CACHE
    if _NC_CACHE is None:
        _NC_CACHE = build()
    return _NC_CACHE


def run(inputs, trace=False, trace_cores=None):
    nc = _get_nc()
    x = np.ascontiguousarray(np.asarray(inputs["input"], dtype=np.float32))
    m = np.ascontiguousarray(np.asarray(inputs["mask"], dtype=np.int32))
    shared = {
        n: np.ascontiguousarray(np.asarray(inputs[n], dtype=np.float32))
        for n in ("W_q", "b_q", "W_k", "b_k", "W_v", "b_v")
    }
    in_maps = [{"input": x[i], "mask": m[i], **shared} for i in range(B)]
    res = run_bass_kernel_spmd(
        nc,
        in_maps,
        core_ids=list(range(B)),
        trace=trace,
        trace_cores=trace_cores,
    )
    out = np.stack([res.results[i]["out"] for i in range(B)])
    return out, res


def kernel(**inputs) -> np.ndarray:
    out, _ = run(inputs, trace=False)
    return out



# revision 4
# speedup vs baseline: 1.6843x; 1.6843x over previous
"""Single-head attention (B=8, S=2048, D=1024, H=64) on 8 TRN2 NeuronCores.

Sharding: data-parallel over batch — one batch element per core, Q/K/V
weights replicated. No collectives; host gathers the 8 per-core outputs.

Host-side layout prep (per core): x fed pre-transposed as xT [D, S] bf16,
mask fed pre-transposed as maskT [S, S] int8, weights fed as W^T bf16 with
W_q/W_k fused into one [D, 128] stationary block.

Per-core pipeline:
  phase 1: qkT [128, S] = (Wqk^T)^T-stationary matmuls over 8 d-chunks
           (q rows 0-63, k rows 64-127), bias via tensor_scalar_add;
           kq_sb [128, S] = partition-swapped copy (SBUF->SBUF DMA) so
           kT also lives at partitions 0-63 and qT at 64-127;
           vT [64, S] similarly, then PE-transposed into v_aug [S,H+1]
           with a ones column (gives softmax denominators for free).
  phase 2: per 512-wide q-block: scoresT [k,q] computed directly
           (kT stationary, qT moving; K=64 row-tiled 2x: pair of k-tiles
           runs concurrently in the two PE row halves), exp via scalar
           ACT (scale=1/8), mask applied as bf16 multiply (maskT int8
           converted once to bf16 on DVE);
           outT[65, q] += v_aug[k-tile].T @ probsT (PSUM accum over k)
           PE-transpose back, multiply by reciprocal of the ones-row,
           DMA out.
"""

import sys
import types

import numpy as np
import ml_dtypes

import concourse.bass as bass
import concourse.mybir as mybir
import concourse.tile as tile
from concourse import bacc
from concourse.bass_utils import run_bass_kernel_spmd
from concourse.masks import make_identity

B, S, D, H = 8, 2048, 1024, 64
NT = S // 128          # 16 k-tiles of 128
NCH = D // 128         # 8 contraction chunks
NB = 4                 # q-blocks of 512

f32 = mybir.dt.float32
bf16 = mybir.dt.bfloat16
i8 = mybir.dt.int8
ACT_EXP = mybir.ActivationFunctionType.Exp


def install_ntff_hook():
    """RL-container antenv stub lacks axon_hooks; inject it so trace=True
    under axon can capture NTFF profiles. Harmless if already present."""
    if "antenv.axon_hooks" in sys.modules:
        return
    try:
        mod = types.ModuleType("antenv.axon_hooks")
        state = {"hook": None}
        mod.set_axon_ntff_profile_hook = lambda h: state.__setitem__("hook", h)
        mod.get_axon_ntff_profile_hook = lambda: state["hook"]
        sys.modules["antenv.axon_hooks"] = mod
        import antenv

        antenv.axon_hooks = mod
        from trn_agent_boot.trn_boot import _ntff_profile_via_ctypes

        mod.set_axon_ntff_profile_hook(
            _ntff_profile_via_ctypes("/opt/axon/libaxon_pjrt.so")
        )
    except Exception:
        pass


def build():
    nc = bacc.Bacc("TRN2", target_bir_lowering=False, debug=False, num_devices=8)

    xT_d = nc.dram_tensor("xT", [D, S], bf16, kind="ExternalInput")
    mT_d = nc.dram_tensor("maskT", [S, S], i8, kind="ExternalInput")
    wqk_d = nc.dram_tensor("wqk", [D, 128], bf16, kind="ExternalInput")
    wv_d = nc.dram_tensor("wv", [D, H], bf16, kind="ExternalInput")
    bqk_d = nc.dram_tensor("bqk", [128], f32, kind="ExternalInput")
    bv_d = nc.dram_tensor("bv", [H], f32, kind="ExternalInput")
    out_d = nc.dram_tensor("out", [S, H], f32, kind="ExternalOutput")

    with tile.TileContext(nc) as tc:
        with (
            tc.tile_pool(name="singles", bufs=1) as singles,
            tc.tile_pool(name="sb", bufs=2) as sb,
            tc.tile_pool(name="mkin", bufs=3) as mkin,
            tc.tile_pool(name="esb", bufs=3) as esb,
            tc.tile_pool(name="pS", bufs=2, space="PSUM") as pS,
            tc.tile_pool(name="pM", bufs=2, space="PSUM") as pM,
            tc.tile_pool(name="pPV", bufs=2, space="PSUM") as pPV,
        ):
            # ---- constants -------------------------------------------------
            id_b = singles.tile([128, 128], bf16)
            make_identity(nc, id_b[:])
            id_f = singles.tile([128, 128], f32)
            make_identity(nc, id_f[:])

            wqk_sb = singles.tile([128, NCH, 128], bf16)
            wv_sb = singles.tile([128, NCH, H], bf16)
            for c in range(NCH):
                nc.sync.dma_start(
                    wqk_sb[:, c, :], wqk_d.ap()[c * 128:(c + 1) * 128, :]
                )
                nc.sync.dma_start(
                    wv_sb[:, c, :], wv_d.ap()[c * 128:(c + 1) * 128, :]
                )
            bqk_sb = singles.tile([128, 1], f32)
            nc.sync.dma_start(
                bqk_sb[:], bass.AP(tensor=bqk_d, offset=0, ap=[[1, 128], [0, 1]])
            )
            bv_sb = singles.tile([H, 1], f32)
            nc.sync.dma_start(
                bv_sb[:], bass.AP(tensor=bv_d, offset=0, ap=[[1, H], [0, 1]])
            )

            # ---- persistent activations -----------------------------------
            xT_sb = singles.tile([128, NCH, S], bf16)
            for c in range(NCH):
                nc.sync.dma_start(
                    xT_sb[:, c, :], xT_d.ap()[c * 128:(c + 1) * 128, :]
                )

            qkT_sb = singles.tile([128, S], bf16)   # q rows 0-63, k rows 64-127
            kq_sb = singles.tile([128, S], bf16)    # k rows 0-63, q rows 64-127
            vT_sb = singles.tile([H, S], bf16)
            v_aug = singles.tile([128, NT, H + 1], bf16)
            nc.gpsimd.memset(v_aug[:, :, H:H + 1], 1.0)

            m_bf = singles.tile([128, NT, S], bf16)  # maskT as bf16

            # ---- mask load + convert (independent of compute) --------------
            for kt in range(NT):
                m_i8 = mkin.tile([128, S], i8, tag="mi")
                nc.sync.dma_start(m_i8[:], mT_d.ap()[kt * 128:(kt + 1) * 128, :])
                nc.vector.tensor_copy(m_bf[:, kt, :], m_i8[:])

            # ---- phase 1: projections --------------------------------------
            for blk in range(NB):
                sl = slice(blk * 512, (blk + 1) * 512)
                qk_ps = pS.tile([128, 512], f32, tag="S")
                for c in range(NCH):
                    nc.tensor.matmul(
                        qk_ps[:],
                        wqk_sb[:, c, :],
                        xT_sb[:, c, sl],
                        start=(c == 0),
                        stop=(c == NCH - 1),
                    )
                nc.vector.tensor_scalar_add(qkT_sb[:, sl], qk_ps[:], bqk_sb[:])
                # partition swap: kT to rows 0-63, qT duplicate to rows 64-127
                nc.sync.dma_start(kq_sb[0:64, sl], qkT_sb[64:128, sl])
                nc.sync.dma_start(kq_sb[64:128, sl], qkT_sb[0:64, sl])

                vT_ps = pM.tile([H, 512], f32, tag="M")
                for c in range(NCH):
                    nc.tensor.matmul(
                        vT_ps[:],
                        wv_sb[:, c, :],
                        xT_sb[:, c, sl],
                        start=(c == 0),
                        stop=(c == NCH - 1),
                    )
                nc.vector.tensor_scalar_add(vT_sb[:, sl], vT_ps[:], bv_sb[:])

            for t in range(NT):
                vtr_ps = pM.tile([128, H], bf16, tag="M")
                nc.tensor.transpose(
                    vtr_ps[:], vT_sb[:, t * 128:(t + 1) * 128], id_b[0:H, 0:H]
                )
                nc.scalar.copy(v_aug[:, t, 0:H], vtr_ps[:])

            # ---- phase 2: attention ----------------------------------------
            for blk in range(NB):
                qsl = slice(blk * 512, (blk + 1) * 512)
                probsT = sb.tile([128, NT, 512], bf16, tag="pT")
                for kd in range(NT // 2):   # pairs of k-tiles, row-tiled 2x
                    ka, kb = 2 * kd, 2 * kd + 1
                    sc_ps = pS.tile([128, 1024], f32, tag="S")
                    # row group 0-63: kT/qT live at partitions 0-63
                    nc.tensor.matmul(
                        sc_ps[:, 0:512],
                        kq_sb[0:64, ka * 128:(ka + 1) * 128],
                        qkT_sb[0:64, qsl],
                        start=True,
                        stop=True,
                    )
                    # row group 64-127: kT at qkT rows 64-127, qT at kq rows 64-127
                    nc.tensor.matmul(
                        sc_ps[:, 512:1024],
                        qkT_sb[64:128, kb * 128:(kb + 1) * 128],
                        kq_sb[64:128, qsl],
                        start=True,
                        stop=True,
                    )
                    e_sb = esb.tile([128, 1024], bf16, tag="e")
                    nc.scalar.activation(
                        e_sb[:], sc_ps[:], ACT_EXP, bias=0.0, scale=0.125
                    )
                    for j in range(2):
                        nc.vector.tensor_mul(
                            probsT[:, kd * 2 + j, :],
                            e_sb[:, j * 512:(j + 1) * 512],
                            m_bf[:, kd * 2 + j, qsl],
                        )

                pv_ps = pPV.tile([H + 1, 512], f32, tag="pv")
                for kt in range(NT):
                    nc.tensor.matmul(
                        pv_ps[:],
                        v_aug[:, kt, :],
                        probsT[:, kt, :],
                        start=(kt == 0),
                        stop=(kt == NT - 1),
                    )
                oT_sb = sb.tile([H + 1, 512], f32, tag="oT")
                nc.scalar.copy(oT_sb[:], pv_ps[:])

                for qq in range(4):
                    qt = blk * 4 + qq
                    o2_ps = pM.tile([128, H + 1], f32, tag="M")
                    nc.tensor.transpose(
                        o2_ps[:],
                        oT_sb[:, qq * 128:(qq + 1) * 128],
                        id_f[0:H + 1, 0:H + 1],
                    )
                    rcp = sb.tile([128, 1], f32, tag="rcp")
                    nc.vector.reciprocal(rcp[:], o2_ps[:, H:H + 1])
                    out_sb = sb.tile([128, H], f32, tag="osb")
                    nc.vector.tensor_scalar_mul(out_sb[:], o2_ps[:, 0:H], rcp[:])
                    nc.sync.dma_start(
                        out_d.ap()[qt * 128:(qt + 1) * 128, :], out_sb[:]
                    )

    nc.compile()
    return nc


_NC_CACHE = None


def _get_nc():
    global _NC_CACHE
    if _NC_CACHE is None:
        _NC_CACHE = build()
    return _NC_CACHE


def _prep_core_inputs(inputs):
    x = np.asarray(inputs["input"], dtype=np.float32)
    m = np.asarray(inputs["mask"])
    W_q = np.asarray(inputs["W_q"], dtype=np.float32)
    W_k = np.asarray(inputs["W_k"], dtype=np.float32)
    W_v = np.asarray(inputs["W_v"], dtype=np.float32)
    wqk = np.concatenate([W_q.T, W_k.T], axis=1).astype(ml_dtypes.bfloat16)
    wv = np.ascontiguousarray(W_v.T).astype(ml_dtypes.bfloat16)
    bqk = np.concatenate(
        [np.asarray(inputs["b_q"]), np.asarray(inputs["b_k"])]
    ).astype(np.float32)
    bv = np.asarray(inputs["b_v"], dtype=np.float32)
    shared = {"wqk": wqk, "wv": wv, "bqk": bqk, "bv": bv}
    in_maps = []
    for i in range(B):
        in_maps.append(
            {
                "xT": x[i].T.astype(ml_dtypes.bfloat16),
                "maskT": m[i].T.astype(np.int8),
                **shared,
            }
        )
    return in_maps


def run(inputs, trace=False, trace_cores=None):
    nc = _get_nc()
    in_maps = _prep_core_inputs(inputs)
    res = run_bass_kernel_spmd(
        nc,
        in_maps,
        core_ids=list(range(B)),
        trace=trace,
        trace_cores=trace_cores,
    )
    out = np.stack([res.results[i]["out"] for i in range(B)])
    return out, res


def kernel(**inputs) -> np.ndarray:
    out, _ = run(inputs, trace=False)
    return out
